# revision 16
# baseline (speedup 1.0000x reference)
"""Trainium2 Bass kernel v2 for 8x8 block 2D-DCT (nn_DCT2d).

Input : x (32, 1, 1024, 1024) fp32
Output: coeff (32, 16384, 8, 8) fp32 where coeff[n,k] = A @ block_k @ A^T

Two-pass PE design, bf16 internal compute, dense 4KB/partition DMAs:

Per (image n, 128-row chunk R) tile [128 rows x 1024 cols]:
  row r = (h2,h1,h0,b0,i2,i1,i0)   [blockrow g=(h,b0), i = row in block]
  col w = (bj6..bj0,j2,j1,j0)      [bj = blockcol, j = col in block]

  1. load  : SWDGE cast DMA HBM f32 -> SBUF bf16, T0[p=r, f=w] (contig 4KB rows)
  2. MM1   : Y[(g,u), w] = sum_i A[u,i] T0[(g,i), w]   (W1 = blkdiag A, 2x N=512)
  3. copy1 : PSUM f32 -> SBUF bf16 (linear)
  4. PEtr  : per c=bjh3: X2[(bjl,j), (c|h,b0,u)] = Ysb[:, c-slice].T (bf16 psum)
  5. copy2 : PSUM -> SBUF, f-reorder (c,hb,u) -> (u,hb,c)
  6. MM2   : per u: Z[(h,b0,bjh3), (u|bjl,v)] = sum_j X2sb[(bjl,j),(h,b0,c)] A[v,j]
  7. copy3 : PSUM f32 -> SBUF f32, f-reorder (u,bjl,v) -> (bjl,u,v)
  8. store : HWDGE dense DMA, out chunk = [128 part x 4KB contig]
"""
import numpy as np
from contextlib import ExitStack

import concourse.bass as bass
import concourse.tile as tile
from concourse import bacc, mybir
from concourse.bass_utils import run_bass_kernel_spmd
import ml_dtypes

N_CORES = 8
IMGS_PER_CORE = 4  # 32 / 8
F32 = mybir.dt.float32
BF16 = mybir.dt.bfloat16

_BS = 8


def _make_dct_matrix(bs=_BS):
    A = np.zeros((bs, bs), dtype=np.float64)
    for i in range(bs):
        c_i = 1.0 / np.sqrt(2.0) if i == 0 else 1.0
        for n in range(bs):
            A[i, n] = np.sqrt(2.0 / bs) * c_i * np.cos((2 * n + 1) / (bs * 2) * i * np.pi)
    return A.astype(np.float32)


def _make_w1(A):
    """W1[g*8+i, g*8+u] = A[u,i]: 16-block diagonal, vertical DCT."""
    W = np.zeros((128, 128), dtype=np.float32)
    for g in range(16):
        W[g * 8:(g + 1) * 8, g * 8:(g + 1) * 8] = A.T
    return W


def _make_w2(A):
    """W2[q*8+j, q*8+v] = A[v,j]: 16-block diagonal, horizontal DCT."""
    return _make_w1(A)  # same structure: A.T blocks


def build_nc(n_imgs=IMGS_PER_CORE, repeat=1, opts=None, io="external"):
    o = {
        "load": "swdge_cast",    # swdge_cast | hwdge_dve | hwdge_scalar
        "store_eng": "sync",     # sync | scalar | alt
        "copy1_eng": "scalar",   # scalar | vector
        "copy2_eng": "vector",
        "copy3_eng": "scalar",
        "pair": False,           # legacy alias for cpt=2
        "cpt": 2,                # chunks fused per tile (1|2|4): cpt*512KB DMAs
        "bufs": {"t0": 8, "yps": 3, "ysb": 4, "x2ps": 2, "x2sb": 4,
                 "zps": 3, "st": 8, "t0f": 2},
        "ablate": [],   # bench-only: subset of load,mm1,copy1,tr,copy2,mm2,copy3,store
    }
    o.update(opts or {})
    A = _make_dct_matrix()
    w1_np = _make_w1(A).astype(ml_dtypes.bfloat16)
    w2_np = _make_w2(A).astype(ml_dtypes.bfloat16)
    id_np = np.eye(128, dtype=ml_dtypes.bfloat16)

    nc = bacc.Bacc(
        "TRN2",
        target_bir_lowering=False,
        debug=False,
        num_devices=N_CORES,
    )
    kind = "ExternalInput" if io == "external" else "Internal"
    okind = "ExternalOutput" if io == "external" else "Internal"
    x = nc.dram_tensor("x", [n_imgs * 1024, 1024], F32, kind=kind)
    out = nc.dram_tensor("out", [n_imgs * 1048576], F32, kind=okind)
    if io != "external":
        dummy_in = nc.dram_tensor("dummy_in", [128], F32, kind="ExternalInput")
        dummy_out = nc.dram_tensor("dummy_out", [128], F32, kind="ExternalOutput")

    w1_d = nc.inline_tensor(np.ascontiguousarray(w1_np), "w1c")
    w2_d = nc.inline_tensor(np.ascontiguousarray(w2_np), "w2c")
    id_d = nc.inline_tensor(np.ascontiguousarray(id_np), "idc")

    if o["pair"]:
        o["cpt"] = 2
    cpt = o["cpt"]
    xv = x.ap().rearrange("(n R p) w -> n R p w", n=n_imgs, R=8, p=128)
    ov = out.ap().rearrange("(n R p f) -> n R p f", n=n_imgs, R=8, p=128, f=1024)
    # fused views: cpt consecutive chunks side by side on the free dim
    xvC = x.ap().rearrange("(n Rp cc p) w -> n Rp p cc w",
                           n=n_imgs, Rp=8 // cpt, cc=cpt, p=128)
    ovC = out.ap().rearrange("(n Rp cc p f) -> n Rp p cc f",
                             n=n_imgs, Rp=8 // cpt, cc=cpt, p=128, f=1024)

    copy_ops = {
        "scalar": lambda dst, src: nc.scalar.copy(dst, src),
        "vector": lambda dst, src: nc.vector.tensor_copy(dst, src),
    }
    c1 = copy_ops[o["copy1_eng"]]
    c2 = copy_ops[o["copy2_eng"]]
    c3 = copy_ops[o["copy3_eng"]]
    if o["store_eng"] == "alt":
        store_dmas = [nc.sync.dma_start, nc.scalar.dma_start]
    else:
        store_dmas = [getattr(nc, o["store_eng"]).dma_start] * 2

    with tile.TileContext(nc) as tc, ExitStack() as ctx:
        B = dict(o["bufs"])
        for stage, pool in (("mm1", "yps"), ("tr", "x2ps"), ("mm2", "zps")):
            if stage in o["ablate"]:
                B[pool] = 1
        wp = ctx.enter_context(tc.tile_pool(name="w", bufs=1))
        t0p = ctx.enter_context(tc.tile_pool(name="t0", bufs=B["t0"]))
        ypsp = ctx.enter_context(
            tc.tile_pool(name="yps", bufs=B["yps"], space=bass.MemorySpace.PSUM))
        ysbp = ctx.enter_context(tc.tile_pool(name="ysb", bufs=B["ysb"]))
        x2psp = ctx.enter_context(
            tc.tile_pool(name="x2ps", bufs=B["x2ps"], space=bass.MemorySpace.PSUM))
        x2sbp = ctx.enter_context(tc.tile_pool(name="x2sb", bufs=B["x2sb"]))
        zpsp = ctx.enter_context(
            tc.tile_pool(name="zps", bufs=B["zps"], space=bass.MemorySpace.PSUM))
        stp = ctx.enter_context(tc.tile_pool(name="st", bufs=B["st"]))
        t0fp = None
        if o["load"] != "swdge_cast":
            t0fp = ctx.enter_context(tc.tile_pool(name="t0f", bufs=B["t0f"]))

        w1t = wp.tile([128, 128], BF16)
        w2t = wp.tile([128, 128], BF16)
        idt = wp.tile([128, 128], BF16)
        nc.sync.dma_start(w1t[:], w1_d.ap())
        nc.sync.dma_start(w2t[:], w2_d.ap())
        nc.sync.dma_start(idt[:], id_d.ap())

        ab = set(o["ablate"])
        # static dummy sources so ablated producers leave consumers valid
        s_bf = s_f32 = s_ps = s_ps16 = None
        need_ps = ("mm1" in ab and "copy1" not in ab) or (
            "mm2" in ab and "copy3" not in ab)
        need_ps16 = "tr" in ab and "copy2" not in ab
        if ab & {"load", "copy1", "copy2"} or need_ps:
            s_bf = wp.tile([128, 1024], BF16)
            nc.vector.memzero(s_bf[:])
        if "copy3" in ab and "store" not in ab:
            s_f32 = wp.tile([128, 1024 * cpt], F32)
            nc.vector.memzero(s_f32[:])
        if need_ps:
            s_ps = wp.tile([128, 512], F32, space=bass.MemorySpace.PSUM)
            nc.tensor.matmul(s_ps[:], w1t[:], s_bf[:, 0:512],
                             start=True, stop=True)
        if need_ps16:
            s_ps16 = wp.tile([128, 1024], BF16, space=bass.MemorySpace.PSUM)
            for c in range(8):
                nc.tensor.transpose(s_ps16[:, c * 128:(c + 1) * 128],
                                    w1t[:], idt[:])

        def compute_chunk(t0r, coff, st, soff):
            """DCT one [128,1024] chunk: t0 cols [coff,coff+1024) -> st cols
            [soff,soff+1024)."""
            # MM1 + copy1 in halves (one PSUM bank each)
            ysb = ysbp.tile([128, 1024], BF16)
            for half in range(2):
                if "mm1" not in ab:
                    yp = ypsp.tile([128, 512], F32)
                    nc.tensor.matmul(
                        yp[:], w1t[:],
                        t0r[:, coff + half * 512:coff + (half + 1) * 512],
                        start=True, stop=True,
                    )
                ypr = s_ps if "mm1" in ab else yp
                if "copy1" not in ab:
                    c1(ysb[:, half * 512:(half + 1) * 512], ypr[:])
            ysbr = s_bf if "copy1" in ab else ysb

            # PE transposes: one bf16 PSUM bank holds all 8 slices
            if "tr" not in ab:
                x2p = x2psp.tile([128, 1024], BF16)
                for c in range(8):
                    nc.tensor.transpose(
                        x2p[:, c * 128:(c + 1) * 128],
                        ysbr[:, c * 128:(c + 1) * 128],
                        idt[:],
                    )
            x2pr = s_ps16 if "tr" in ab else x2p
            # copy2: f reorder (c, hb, u) -> (u, hb, c)
            x2sb = x2sbp.tile([128, 1024], BF16)
            if "copy2" not in ab:
                c2(x2sb[:].rearrange("p (u hb c) -> p u hb c", u=8, hb=16, c=8),
                   x2pr[:].rearrange("p (c hb u) -> p u hb c", c=8, hb=16, u=8))
            x2sbr = s_bf if "copy2" in ab else x2sb

            # MM2 + copy3 in halves of 4 u-values (one PSUM bank each)
            stv = st[:, soff:soff + 1024].rearrange(
                "p (bjl u v) -> p bjl u v", bjl=16, u=8, v=8)
            for half in range(2):
                if "mm2" not in ab:
                    zp = zpsp.tile([128, 512], F32)
                    for ul in range(4):
                        u = half * 4 + ul
                        nc.tensor.matmul(
                            zp[:, ul * 128:(ul + 1) * 128],
                            x2sbr[:, u * 128:(u + 1) * 128],
                            w2t[:],
                            start=True, stop=True,
                        )
                zpr = s_ps if "mm2" in ab else zp
                if "copy3" not in ab:
                    c3(stv[:, :, half * 4:(half + 1) * 4, :],
                       zpr[:].rearrange("p (u bjl v) -> p bjl u v",
                                        u=4, bjl=16, v=8))

        def load_tile(dst_bf, dst_width, src_ap):
            if o["load"] == "swdge_cast":
                nc.gpsimd.dma_start(dst_bf, src_ap)
                return
            cc = dst_width // 1024
            t0f = t0fp.tile([128, dst_width], F32)
            t0fv = (t0f[:] if cc == 1 else
                    t0f[:].rearrange("p (cc w) -> p cc w", cc=cc))
            nc.sync.dma_start(t0fv, src_ap)
            cast = (nc.vector.tensor_copy if o["load"] == "hwdge_dve"
                    else nc.scalar.copy)
            cast(dst_bf, t0fv)

        def emit_tile(n, R):
            t0 = t0p.tile([128, 1024], BF16)
            if "load" not in ab:
                load_tile(t0[:], 1024, xv[n, R])
            t0r = s_bf if "load" in ab else t0
            st = stp.tile([128, 1024], F32)
            compute_chunk(t0r, 0, st, 0)
            if "store" not in ab:
                str_ = s_f32 if "copy3" in ab else st
                store_dmas[R % 2](ov[n, R], str_[:])

        def emit_fused(n, Rp):
            assert "load" not in ab, "fused mode: load ablation unsupported"
            t0 = t0p.tile([128, 1024 * cpt], BF16)
            load_tile(t0[:].rearrange("p (cc w) -> p cc w", cc=cpt),
                      1024 * cpt, xvC[n, Rp])
            st = stp.tile([128, 1024 * cpt], F32)
            for ch in range(cpt):
                compute_chunk(t0, ch * 1024, st, ch * 1024)
            if "store" not in ab:
                str_ = s_f32 if "copy3" in ab else st
                store_dmas[Rp % 2](
                    ovC[n, Rp], str_[:].rearrange("p (cc f) -> p cc f", cc=cpt))

        for rep in range(repeat):
            for n in range(n_imgs):
                if cpt > 1:
                    for Rp in range(8 // cpt):
                        emit_fused(n, Rp)
                else:
                    for R in range(8):
                        emit_tile(n, R)

        if io != "external":
            dt = stp.tile([1, 128], F32)
            nc.sync.dma_start(dt[:], dummy_in.ap().rearrange("(p f) -> p f", p=1))
            nc.sync.dma_start(dummy_out.ap().rearrange("(p f) -> p f", p=1), dt[:])

    nc.compile()
    return nc


_NC_CACHE = {}


def _get_nc():
    if "nc" not in _NC_CACHE:
        _NC_CACHE["nc"] = build_nc()
    return _NC_CACHE["nc"]


def kernel(x, A=None, **_ignored):
    x = np.ascontiguousarray(np.asarray(x, dtype=np.float32))
    assert x.shape == (32, 1, 1024, 1024), x.shape

    nc = _get_nc()
    xf = x.reshape(32, 1024, 1024)
    in_maps = []
    for c in range(N_CORES):
        shard = xf[c * IMGS_PER_CORE:(c + 1) * IMGS_PER_CORE].reshape(
            IMGS_PER_CORE * 1024, 1024
        )
        in_maps.append({"x": np.ascontiguousarray(shard)})

    res = run_bass_kernel_spmd(nc, in_maps, list(range(N_CORES)))
    outs = [
        res.results[c]["out"].reshape(IMGS_PER_CORE, 16384, 8, 8)
        for c in range(N_CORES)
    ]
    return np.concatenate(outs, axis=0)
